# revision 1
# baseline (speedup 1.0000x reference)
"""BlockSparseRingMultiheadDilatedAttention Trainium2 kernel (v2).

Problem (hardcoded): B=1, N=8192, E=1024, H=16 heads, D=64.
Two dilated groups: g0 = heads 0-7, seg 2048, dilation 1;
                    g1 = heads 8-15, seg 4096, dilation 2, offset 1 (odd positions).
Causal within each (gathered) segment.

Sharding over 8 cores (uniform SPMD program, per-core data):
  core c: a = c%2, sc = c//2, b = c%4, rc = c//4
    g0: seg sc (rows 2048*sc .. +2048), heads 4a..4a+4   (4 blocks of [2048 x 2048])
    g1: seg rc odd rows (gathered, 2048 rows), heads 8+2b..+2 (2 blocks)
  Host pre-slices inputs (bf16 cast, odd-row gather, weight head slices) so the
  device program is identical on every core.  Host sums the per-core partial
  output projections (disjoint head contributions) and adds bo + the bv
  pass-through term (softmax rows sum to 1, so bv adds linearly after).

v2 device dataflow per core (vs v1 baseline):
  - S^T score matmuls for the 2 heads of a pair are row-tiled (K=64 at
    tile_position (0,0)/(64,0)) and emitted adjacently -> run concurrently.
    s tile [128,1024] = kpos chunk i for BOTH heads (bank-aligned halves).
  - Causal trimming: diagonal chunks only compute/exp/mask columns >= trim.
  - One exp per chunk over both heads via a [128,2,W] strided AP.
  - Single triangle mask constant, applied to the 4 diagonal chunks per
    (j, pair) with a DVE multiply.
  - V projected directly in natural layout (x chunk as stationary operand),
    no PE transposes / per-chunk DVE copies.
  - reciprocal_approx_fast for softmax denominators (5x faster than
    nc.vector.reciprocal).
  - y0/y1 stored bf16 (halves output DMA).
  - Emission interleaves pair-2 projections and output projections into the
    ACT-paced attention phases so the PE stays busy.
"""

import numpy as np
import ml_dtypes

BF16 = ml_dtypes.bfloat16

SEG = 2048          # rows per attention block (both groups, post-gather)
E = 1024            # embedding
NQ = 512            # tq chunk (one PSUM bank of fp32)
NTQ = SEG // NQ     # 4 tq chunks per block
NTK = SEG // 128    # 16 tk chunks per block
ECH = E // 128      # 8 embedding chunks
VSP = 66            # per-head stride in the vn tile (64 v dims + ones + pad)

_CACHE = {}


def _build_program():
    import concourse.bacc as bacc
    import concourse.mybir as mybir
    import concourse.tile as tile

    dt = mybir.dt
    nc = bacc.Bacc("TRN2", target_bir_lowering=False, debug=False,
                   enable_asserts=False)

    # ---- DRAM I/O (uniform across cores; host slices per core) ----
    # x/w pre-packed by the host in SBUF layout: row p = concat over ec of
    # x[128*ec + p, :] -> 32KB contiguous DMA rows (descriptor-rate matters)
    xs = {}
    for sel in ("a", "b"):      # a = g0 rows, b = g1 gathered odd rows
        for inp in ("q", "k", "v"):
            xs[(sel, inp)] = nc.dram_tensor(
                f"x{sel}_{inp}", [128, ECH * SEG], dt.bfloat16,
                kind="ExternalInput").ap()
    ws = {inp: nc.dram_tensor(f"w{inp}", [128, ECH * 384], dt.bfloat16,
                              kind="ExternalInput").ap()
          for inp in ("q", "k", "v")}
    wo = nc.dram_tensor("wo", [384, E], dt.bfloat16, kind="ExternalInput").ap()
    bs = {inp: nc.dram_tensor(f"b{inp}", [384, 1], dt.float32,
                              kind="ExternalInput").ap()
          for inp in ("q", "k")}
    y0 = nc.dram_tensor("y0", [SEG, E], dt.bfloat16, kind="ExternalOutput").ap()
    y1 = nc.dram_tensor("y1", [SEG, E], dt.bfloat16, kind="ExternalOutput").ap()

    with tile.TileContext(nc) as tc:
        from contextlib import ExitStack
        with ExitStack() as ctx:
            const = ctx.enter_context(tc.tile_pool(name="const", bufs=1))
            wpool = ctx.enter_context(tc.tile_pool(name="wpool", bufs=1))
            xtp = ctx.enter_context(tc.tile_pool(name="xtp", bufs=3))
            qkt = ctx.enter_context(tc.tile_pool(name="qkt", bufs=1))
            vnat = ctx.enter_context(tc.tile_pool(name="vnat", bufs=1))
            otp = ctx.enter_context(tc.tile_pool(name="otp", bufs=1))
            ptp = ctx.enter_context(tc.tile_pool(name="ptp", bufs=3))
            smallp = ctx.enter_context(tc.tile_pool(name="smallp", bufs=4))
            ypool = ctx.enter_context(tc.tile_pool(name="ypool", bufs=3))
            ps_mm = ctx.enter_context(
                tc.tile_pool(name="ps_mm", bufs=2, space="PSUM"))
            ps_acc = ctx.enter_context(
                tc.tile_pool(name="ps_acc", bufs=4, space="PSUM"))

            # ---- warm-up: preload the exp ACT table during the DMA wait ----
            warm = const.tile([1, 1], dt.float32, tag="warm")
            nc.gpsimd.memset(warm, 1.0)
            nc.scalar.activation(warm, warm,
                                 mybir.ActivationFunctionType.Exp,
                                 bias=0.0, scale=1.0)
            # ones row for the PE rank-1 broadcast of 1/d
            onesr = const.tile([1, 128], dt.float32, tag="onesr")
            nc.gpsimd.memset(onesr, 1.0)

            # ---- constants: triangle mask (shared by both heads/halves) ----
            # maskC[p, 512*h + w] = 1.0 if w >= p else 0.0
            maskC = const.tile([128, 2 * NQ], dt.bfloat16, tag="maskC")
            nc.gpsimd.memset(maskC, 1.0)
            nc.gpsimd.affine_select(
                out=maskC.rearrange("p (h w) -> p h w", h=2),
                in_=maskC.rearrange("p (h w) -> p h w", h=2),
                compare_op=mybir.AluOpType.is_ge,
                fill=0.0, base=0, pattern=[[0, 2], [1, NQ]],
                channel_multiplier=-1)

            # ---- weights (host-pre-transposed; plain DMA loads) ----
            # wTa[inp]: [128 e, ECH*384]; chunk ec pair p at 384*ec + 128*p
            wTa = {}
            for inp in ("q", "k", "v"):
                t = wpool.tile([128, 384 * ECH], dt.bfloat16,
                               tag=f"wT_{inp}", name=f"wT_{inp}")
                nc.sync.dma_start(out=t, in_=ws[inp])
                wTa[inp] = t
            # wT[inp][p]: 3D view [128 e, ECH, 128 d]; chunk ec = [:, ec, :]
            wT = {inp: [wTa[inp].rearrange("p (ec x) -> p ec x", x=384)
                        [:, :, 128 * p:128 * (p + 1)]
                        for p in range(3)] for inp in ("q", "k", "v")}
            # woT[p]: [128 (2 heads d), 1024 j]
            woT = []
            for p in range(3):
                t = wpool.tile([128, E], dt.bfloat16, tag=f"woT_{p}")
                nc.sync.dma_start(out=t, in_=wo[128 * p:128 * (p + 1), :])
                woT.append(t)
            # biases -> SBUF [128,1] per (inp, pair), q/k only
            bsb = {}
            for inp in ("q", "k"):
                for p in range(3):
                    t = wpool.tile([128, 1], dt.float32, tag=f"b_{inp}_{p}")
                    nc.sync.dma_start(
                        out=t, in_=bs[inp][128 * p:128 * (p + 1), :])
                    bsb[(inp, p)] = t

            # ---- persistent per-pair activations ----
            qT = [qkt.tile([128, SEG], dt.bfloat16, tag=f"qT{p}", name=f"qT{p}")
                  for p in range(3)]
            kT = [qkt.tile([128, SEG], dt.bfloat16, tag=f"kT{p}", name=f"kT{p}")
                  for p in range(3)]
            # V natural: per pair [128, NTK*2*VSP]; chunk i head h lhsT (65
            # cols: 64 v dims + ones) = [:, 2*VSP*i + VSP*h : +65]
            vn = [vnat.tile([128, NTK * 2 * VSP], dt.bfloat16,
                            tag=f"vn{p}", name=f"vn{p}")
                  for p in range(3)]
            oT = [otp.tile([128, SEG], dt.bfloat16, tag=f"oT{p}", name=f"oT{p}")
                  for p in range(3)]

            # ---- projection emitters (generators yielding PE quanta) ----
            def load_xt(sel, inp, gate_src=None):
                xt = xtp.tile([128, ECH * SEG], dt.bfloat16, tag="xt")
                if gate_src is not None:
                    # artificial WAW gate: delays these DMAs until gate_src
                    # is written, so earlier loads get full DMA bandwidth
                    nc.gpsimd.tensor_copy(xt[:, 0:1], gate_src)
                for c in range(2):
                    w = ECH * SEG // 2
                    nc.sync.dma_start(
                        out=xt[:, w * c:w * (c + 1)],
                        in_=xs[(sel, inp)][:, w * c:w * (c + 1)])
                return xt

            def proj_qk(xt, inp, p):
                """Emit projection of one input tensor into qT/kT[p].
                [128x512] groups on ps_acc so s-tiles keep their pool."""
                dst = (qT if inp == "q" else kT)[p]
                for g in range(NTQ):
                    acc = ps_acc.tile([128, NQ], dt.float32, tag="acc")
                    for ec in range(ECH):
                        nc.tensor.matmul(
                            acc,
                            wT[inp][p][:, ec, :],
                            xt[:, SEG * ec + NQ * g:SEG * ec + NQ * (g + 1)],
                            start=(ec == 0), stop=(ec == ECH - 1))
                        if ec % 4 == 3:
                            yield
                    nc.scalar.activation(
                        dst[:, NQ * g:NQ * (g + 1)], acc,
                        mybir.ActivationFunctionType.Identity,
                        bias=bsb[(inp, p)], scale=1.0)

            def proj_v(xt, p):
                """Emit V-natural projection into vn[p] (+ ones columns)."""
                # rank-3 views only (keep APs simple for lowering)
                vv = vn[p].rearrange("p (kh y) -> p kh y", y=VSP)
                nc.gpsimd.memset(vv[:, :, 64:65], 1.0)
                vk = vn[p].rearrange("p (k y) -> p k y", y=2 * VSP)
                for g in range(4):          # 4 kpos chunks per accv tile
                    accv = ps_acc.tile([128, NQ], dt.float32, tag="acc")
                    for kbs in range(4):
                        kb = 4 * g + kbs
                        for ec in range(ECH):
                            nc.tensor.matmul(
                                accv[:, 128 * kbs:128 * (kbs + 1)],
                                xt[:, SEG * ec + 128 * kb:
                                   SEG * ec + 128 * (kb + 1)],
                                wT["v"][p][:, ec, :],
                                start=(ec == 0), stop=(ec == ECH - 1))
                        yield
                    av = accv.rearrange("p (k y) -> p k y", y=128)
                    for h in range(2):
                        nc.vector.tensor_copy(
                            vk[:, 4 * g:4 * (g + 1),
                               VSP * h:VSP * h + 64],
                            av[:, :, 64 * h:64 * h + 64])
                    yield

            def out_proj(ydram, pairs, ms):
                """Emit output-projection chunks for row-chunks in ms.
                Per-jc [128x512] groups on ps_acc; one DMA per m-chunk."""
                for m in ms:
                    ysb = ypool.tile([128, 2 * NQ], dt.bfloat16, tag="ysb")
                    for jc in range(2):
                        accy = ps_acc.tile([128, NQ], dt.float32, tag="acc",
                                           name="accy")
                        for idx, p in enumerate(pairs):
                            nc.tensor.matmul(
                                accy,
                                oT[p][:, 128 * m:128 * (m + 1)],
                                woT[p][:, NQ * jc:NQ * (jc + 1)],
                                start=(idx == 0), stop=(idx == len(pairs) - 1))
                        nc.vector.tensor_copy(
                            ysb[:, NQ * jc:NQ * (jc + 1)], accy)
                        yield
                    nc.sync.dma_start(
                        out=ydram[128 * m:128 * (m + 1), :], in_=ysb)

            def drain(gen):
                if gen is not None:
                    for _ in gen:
                        pass

            class OutProjFiller:
                """Emits one out-proj row-chunk per pull, gated so chunk m
                only needs oT columns normalized through j = m//4."""

                def __init__(self, ydram, pairs, lead=None):
                    self.gen = out_proj(ydram, pairs, range(NTK))
                    self.m = 0
                    self.lead = lead        # ungated generator to drain first

                def pull(self, j):
                    if self.lead is not None:
                        if next(self.lead, StopIteration) is not StopIteration:
                            return
                        self.lead = None
                    if self.m < NTK and self.m // 4 <= j - 1:
                        if next(self.gen, StopIteration) is not StopIteration:
                            self.m += 1

                def finish(self):
                    drain(self.lead)
                    self.lead = None
                    drain(self.gen)

                def remainder(self):
                    if self.lead is not None:
                        yield from self.lead
                        self.lead = None
                    yield from self.gen

            # ---- attention for one pair, with filler interleaving ----
            def attention_pair(p, filler=None):
                def pull(j):
                    if filler is None:
                        return
                    if hasattr(filler, "pull"):
                        filler.pull(j)
                    else:
                        next(filler, None)

                for j in range(NTQ):
                    nchunks = 4 * (j + 1)
                    acc = [ps_acc.tile([128, NQ], dt.float32, tag="acc",
                                       name=f"acc{h}") for h in range(2)]
                    pend = []   # (i, trim, pt) awaiting O emission

                    def s_chunk(i):
                        """Row-tiled S matmuls + exp (+ mask) for chunk i."""
                        trim = max(0, 128 * i - NQ * j)
                        s = ps_mm.tile([128, 2 * NQ], dt.float32,
                                       tag="mm", name="s")
                        for h in range(2):
                            hp = 64 * h
                            nc.tensor.matmul(
                                s[:, NQ * h + trim:NQ * (h + 1)],
                                kT[p][hp:hp + 64, 128 * i:128 * (i + 1)],
                                qT[p][hp:hp + 64,
                                      NQ * j + trim:NQ * (j + 1)],
                                start=True, stop=True)
                        pt = ptp.tile([128, 2 * NQ], dt.bfloat16,
                                      tag="pt", name="pt")
                        sv = s.rearrange("p (h w) -> p h w", h=2)
                        pv = pt.rearrange("p (h w) -> p h w", h=2)
                        nc.scalar.activation(
                            pv[:, :, trim:NQ], sv[:, :, trim:NQ],
                            mybir.ActivationFunctionType.Exp,
                            bias=0.0, scale=0.125)
                        if trim > 0 or i == 4 * j:   # diagonal chunk
                            mv = maskC.rearrange("p (h w) -> p h w", h=2)
                            nc.vector.tensor_mul(
                                pv[:, :, trim:NQ], pv[:, :, trim:NQ],
                                mv[:, :, 0:NQ - trim])
                        return trim, pt

                    def o_chunk(i, trim, pt):
                        pv = pt.rearrange("p (h w) -> p h w", h=2)
                        for h in range(2):
                            nc.tensor.matmul(
                                acc[h][0:65, trim:NQ],
                                vn[p][:, 2 * VSP * i + VSP * h:
                                      2 * VSP * i + VSP * h + 65],
                                pv[:, h, trim:NQ],
                                start=(i == 0), stop=(i == nchunks - 1),
                                skip_group_check=True)

                    for i in range(nchunks):
                        pend.append((i, *s_chunk(i)))
                        if len(pend) > 1:
                            o_chunk(*pend.pop(0))
                            pull(j)
                    o_chunk(*pend.pop(0))
                    pull(j)

                    # normalize: copy acc out early (frees the psum bank),
                    # then 1/d + broadcast + in-place scale off that path.
                    for h in range(2):
                        hp = 64 * h
                        ov = oT[p][hp:hp + 64, NQ * j:NQ * (j + 1)]
                        dj = smallp.tile([1, NQ], dt.float32, tag="dj",
                                         name="dj")
                        nc.vector.tensor_copy(dj, acc[h][64:65, :])
                        nc.vector.tensor_copy(ov, acc[h][0:64, :])
                        rj = smallp.tile([1, NQ], dt.float32, tag="rj",
                                         name="rj")
                        nc.vector.reciprocal_approx_fast(out=rj, in_=dj)
                        rbps = ps_acc.tile([128, NQ], dt.float32, tag="acc",
                                           name="rbps")
                        nc.tensor.matmul(rbps, onesr, rj,
                                         start=True, stop=True)
                        nc.vector.tensor_mul(ov, ov, rbps[hp:hp + 64, :])
                    pull(j)

            # ================= emission schedule =================
            # Head: staged loads (q alone first, then k gated on q-proj,
            # v gated on q-proj pair 1) interleaved with projections.
            xt_q = load_xt("a", "q")
            drain(proj_qk(xt_q, "q", 0))
            xt_k = load_xt("a", "k", gate_src=qT[0][:, 0:1])
            drain(proj_qk(xt_q, "q", 1))
            drain(proj_qk(xt_k, "k", 0))
            xt_v = load_xt("a", "v", gate_src=kT[0][:, 0:1])
            drain(proj_qk(xt_k, "k", 1))
            drain(proj_v(xt_v, 0))
            drain(proj_v(xt_v, 1))

            # attn(0): pair-2 q/k projections as filler
            def proj_b_qk_gen():
                for inp in ("q", "k"):
                    xt = load_xt("b", inp)
                    yield from proj_qk(xt, inp, 2)

            f0 = proj_b_qk_gen()
            attention_pair(0, filler=f0)
            drain(f0)

            # attn(1): pair-2 V projection, then y0 out-proj chunks (gated,
            # delayed by one j so the PE never waits on a fresh normalize).
            def proj_b_v_gen():
                xt = load_xt("b", "v")
                yield from proj_v(xt, 2)

            f1 = OutProjFiller(y0, (0, 1), lead=proj_b_v_gen())
            attention_pair(1, filler=f1)

            # attn(2): y1 out-proj gated; leftover y0 chunks drain first.
            f2 = OutProjFiller(y1, (2,), lead=f1.remainder())
            attention_pair(2, filler=f2)
            f1.finish()
            f2.finish()

    nc.compile()
    return nc


def _get_program():
    if "nc" not in _CACHE:
        _CACHE["nc"] = _build_program()
    return _CACHE["nc"]


def _prep_inputs(query, key, value, Wq, bq, Wk, bk, Wv, bv, Wo, bo):
    """Build the 8 per-core input maps (host-side slicing + bf16 cast)."""
    q = np.asarray(query, np.float32).reshape(8192, 1024).astype(BF16)
    k = np.asarray(key, np.float32).reshape(8192, 1024).astype(BF16)
    v = np.asarray(value, np.float32).reshape(8192, 1024).astype(BF16)
    wq = np.asarray(Wq, np.float32).astype(BF16)
    wk = np.asarray(Wk, np.float32).astype(BF16)
    wv = np.asarray(Wv, np.float32).astype(BF16)
    wo_f = np.asarray(Wo, np.float32).astype(BF16)
    bqf = np.asarray(bq, np.float32)
    bkf = np.asarray(bk, np.float32)

    qT, kT, vT = q.T, k.T, v.T  # [1024, 8192] views

    def pack_x(xTslice):
        # [1024, 2048] -> SBUF layout [128, ECH*2048] (32KB rows)
        return np.ascontiguousarray(
            np.asarray(xTslice).reshape(ECH, 128, SEG)
            .transpose(1, 0, 2).reshape(128, ECH * SEG))

    def pack_w(wslice):
        # [1024, 384] -> [128, ECH*384]
        return np.ascontiguousarray(
            np.asarray(wslice).reshape(ECH, 128, 384)
            .transpose(1, 0, 2).reshape(128, ECH * 384))

    in_maps = []
    for c in range(8):
        a, sc, b, rc = c % 2, c // 2, c % 4, c // 4
        rows_g0 = slice(2048 * sc, 2048 * (sc + 1))
        rows_g1 = slice(4096 * rc + 1, 4096 * (rc + 1), 2)
        hrows = np.r_[256 * a:256 * a + 256, 512 + 128 * b:512 + 128 * b + 128]
        m = {
            "xa_q": pack_x(qT[:, rows_g0]),
            "xa_k": pack_x(kT[:, rows_g0]),
            "xa_v": pack_x(vT[:, rows_g0]),
            "xb_q": pack_x(qT[:, rows_g1]),
            "xb_k": pack_x(kT[:, rows_g1]),
            "xb_v": pack_x(vT[:, rows_g1]),
            "wq": pack_w(wq[hrows].T),
            "wk": pack_w(wk[hrows].T),
            "wv": pack_w(wv[hrows].T),
            "wo": np.ascontiguousarray(wo_f[:, hrows].T),
            "bq": np.ascontiguousarray(bqf[hrows]).reshape(384, 1),
            "bk": np.ascontiguousarray(bkf[hrows]).reshape(384, 1),
        }
        in_maps.append(m)
    return in_maps


def _combine(results, Wo, bv, bo):
    y = np.zeros((8192, 1024), np.float32)
    for c in range(8):
        sc, rc = c // 2, c // 4
        y[2048 * sc:2048 * (sc + 1)] += np.asarray(
            results[c]["y0"], np.float32)
        y[4096 * rc + 1:4096 * (rc + 1):2] += np.asarray(
            results[c]["y1"], np.float32)
    # bv pass-through: softmax rows sum to 1, so v-bias adds linearly.
    bvf = np.asarray(bv, np.float32)
    if np.any(bvf):
        wof = np.asarray(Wo, np.float32)
        y += bvf[:512] @ wof[:, :512].T                # g0 heads: all rows
        y[1::2] += bvf[512:] @ wof[:, 512:].T          # g1 heads: odd rows
    y += np.asarray(bo, np.float32)
    return y.reshape(1, 8192, 1024)


def kernel(query, key, value, Wq, bq, Wk, bk, Wv, bv, Wo, bo,
           _trace=False, _trace_cores=None):
    from concourse import bass_utils
    nc = _get_program()
    in_maps = _prep_inputs(query, key, value, Wq, bq, Wk, bk, Wv, bv, Wo, bo)
    res = bass_utils.run_bass_kernel_spmd(
        nc, in_maps, core_ids=list(range(8)),
        trace=_trace, trace_cores=_trace_cores)
    _CACHE["last_results"] = res
    return _combine(res.results, Wo, bv, bo)

